# revision 36
# baseline (speedup 1.0000x reference)
"""Distributed causal multi-head attention + output projection for TRN2 (8 NeuronCores).

Problem: q,k,v [4, 2048, 1024] f32, W [1024, 1024], b zeros, mask zeros (no padding).
  out = proj(softmax(causal(q@k.T/8)) @ v) @ W.T + b

Sharding: head-parallel attention + token-parallel projection, glued by 8-way
AllToAll exchanges of the attention outputs (bf16).
  - Core c computes attention for heads {2c, 2c+1} over all 4 batches
    (8 (batch, head) units/core, identical causal structure on every core -> SPMD-uniform).
  - Core j projects the 1024 tokens {batch j//2, q-tiles 4qb+2*(j%2)+{0,1} for qb 0..3}.
  - Sweeps DESCEND qb (3..0): the collective subsystem's ~65us entry barrier
    (NEFF launch skew + ncfw init — no collective completes before ~70us)
    hides under the two big sweeps, and all bulk input DMA lands during sweep
    3's compute. One full exchange per sweep (each isolated collective pays
    ~8-10us of entry/exit overhead, so fewer/bigger ops beat split halves); a
    tiny warm-up collective absorbs the cold-firmware first-op penalty (~5x).
    Chunk qb's at-load+normalize run mid-sweep qb-1, its projection late in
    that sweep; only the last chunk's ~10us exchange + projection is
    tail-exposed, with the penultimate chunk's projection filling the flight.

Queue discipline (the decisive factor — each hardware DMA queue serializes):
  SP carries only the latency-critical stage DMAs + output stores; every bulk
  or collective-gated transfer (input round-2, W, chunk at-loads, denominator
  broadcasts) issues on the GPSIMD SWDGE queue, so nothing head-of-line-blocks
  the stage DMAs that gate exchange triggers.

Dataflow per unit/q-block:
  QK on PE (k-chunk stationary, q moving 512-wide) -> exp on ScalarE (PSUM
  source, causal tiles trimmed; ScalarE is the bottleneck engine at ~165us) ->
  diagonal-tile multiplicative mask on DVE -> AV on PE with V STATIONARY
  (output [dh+1, 512] in PSUM, ONE accumulation group per q-block — vs
  attention-stationary AV this halves matmul count and cuts 1088 LDWEIGHTS;
  ones-column in v gives the softmax denominator as row 64) -> DVE copy to
  SBUF bf16 -> single stage DMA into the exchange buffer in [feat, tok]
  layout (denominator row included).
  The receiver loads [feat, (src, tok)] tiles with PLAIN DMAs (no transposes
  anywhere — payload is already feature-major), reciprocals the 16 denominator
  rows in one DVE op, replicates them across partitions via a 0-stride-AP DMA
  through a DRAM bounce, normalizes with one tensor_tensor per chunk, and
  runs the projection (at stationary, W moving 512-wide).
"""

import sys

sys.path.insert(0, "/opt/trn_rl_repo")

import numpy as np
import ml_dtypes

import concourse.bass as bass  # noqa: F401
import concourse.mybir as mybir
from concourse import bacc
from concourse.bass_utils import run_bass_kernel_spmd
from concourse.tile import TileContext
from concourse.masks import make_upper_triangular
from bass_rust import add_dep_helper

B, S, D, H, DH = 4, 2048, 1024, 16, 64
P = 128
NCORES = 8
UNITS = 8          # (batch, local head) pairs per core
QBLK = 512         # q columns per score block
NQB = S // QBLK    # 4
NKC = S // P       # 16 key chunks
TOK = (B * S) // NCORES  # 1024 tokens projected per core
CROWS = 256        # token rows per core per exchange chunk

# Descending: the ~65us collective-subsystem entry barrier (NEFF launch
# barrier + ncfw init — no collective completes before ~70us) hides under the
# two big sweeps, and all input DMA traffic lands during sweep 3's 60us+ of
# compute, leaving the second half's exchanges an uncontended fabric.
SWEEP_ORDER = [3, 2, 1, 0]
UNIT_ORDER = [0, 2, 4, 6, 1, 3, 5, 7]  # evens feed half-exchange A, odds B

BF16 = ml_dtypes.bfloat16

_CACHE = {}


def _build():
    bf = mybir.dt.bfloat16
    f32 = mybir.dt.float32
    Exp = mybir.ActivationFunctionType.Exp

    nc = bacc.Bacc("TRN2", target_bir_lowering=False, debug=False, num_devices=NCORES)

    kT_ext = nc.declare_dram_parameter("kTz", [UNITS, P, S], bf, isOutput=False)
    qT_ext = nc.declare_dram_parameter("qT", [UNITS // 2, P, S], bf, isOutput=False)
    v_ext = nc.declare_dram_parameter("v", [UNITS, P, NKC * (DH + 1)], bf, isOutput=False)
    wT_ext = nc.declare_dram_parameter("wT", [D, D], bf, isOutput=False)
    out_ext = nc.declare_dram_parameter("out", [TOK, D], f32, isOutput=True)

    with TileContext(nc) as tc:
        with (
            tc.tile_pool(name="const", bufs=1) as constp,
            tc.tile_pool(name="q", bufs=1) as qp,
            tc.tile_pool(name="k", bufs=1) as kp,
            tc.tile_pool(name="v", bufs=1) as vp,
            tc.tile_pool(name="attn", bufs=22) as attnp,
            tc.tile_pool(name="avs", bufs=10) as avsp,
            tc.tile_pool(name="atc", bufs=2) as atcp,
            tc.tile_pool(name="atn", bufs=2) as atnp,
            tc.tile_pool(name="dr", bufs=2) as drp,
            tc.tile_pool(name="den", bufs=2) as denp,
            tc.tile_pool(name="w", bufs=1) as wp,
            tc.tile_pool(name="osb", bufs=2) as osb,
            tc.tile_pool(name="dram", bufs=1, space="DRAM") as dramp,
            tc.tile_pool(name="pscore", bufs=2, space="PSUM") as pscore,
            tc.tile_pool(name="pav", bufs=2, space="PSUM") as pav,
            tc.tile_pool(name="pproj", bufs=2, space="PSUM") as pproj,
        ):
            # Multiplicative causal mask for diagonal tiles, [k, q] layout:
            # m01[kk, qq] = 1.0 iff qq >= kk.
            m01 = constp.tile([P, P], bf)
            make_upper_triangular(nc, m01[:], val=1.0, diag=True)

            # Resident q/k/v in fused tiles (unit on a free dim). Sweep qb=3
            # needs unit u's FULL k/v and q columns 1536:2048, so load
            # per-unit in sweep unit order: the first unit's tensors on the
            # SP queue (compute starts ~3us in), everything else on the
            # GPSIMD SWDGE queue so the ~10MB of bulk transfers never
            # serialize ahead of stage DMAs on the SP hardware queue.
            k_all = kp.tile([P, UNITS, S], bf)
            q_all = qp.tile([P, B, S], bf)
            v_all = vp.tile([P, UNITS, NKC, DH + 1], bf)
            kT_r = kT_ext.ap().rearrange("u p s -> p u s")
            qT_r = qT_ext.ap().rearrange("b p s -> p b s")
            v_r = v_ext.ap().rearrange("u p (c d) -> p u c d", d=DH + 1)
            QT = (NQB - 1) * QBLK
            u0 = UNIT_ORDER[0]
            nc.sync.dma_start(k_all[:, u0, :QBLK], kT_r[:, u0, :QBLK])
            nc.sync.dma_start(q_all[:, :, QT:], qT_r[:, :, QT:])
            nc.sync.dma_start(v_all[:, u0, 0:4, :], v_r[:, u0, 0:4, :])
            nc.sync.dma_start(k_all[:, u0, QBLK:], kT_r[:, u0, QBLK:])
            nc.sync.dma_start(v_all[:, u0, 4:, :], v_r[:, u0, 4:, :])
            # Tiny warm-up collective, triggered before the bulk round-2
            # issues occupy the GPSIMD queue: the first collective after the
            # entry barrier otherwise runs ~5x slower (cold firmware path),
            # which in run timings turned a 10us exchange into 52us.
            a2a_wi = dramp.tile([NCORES, 64], bf, name="a2a_wi", tag="a2a_wi")
            a2a_wo = dramp.tile([NCORES, 64], bf, name="a2a_wo", tag="a2a_wo")
            nc.gpsimd.collective_compute(
                "AllToAll",
                mybir.AluOpType.bypass,
                replica_groups=[list(range(NCORES))],
                ins=[a2a_wi.opt()],
                outs=[a2a_wo.opt()],
            )
            for u in UNIT_ORDER[1:]:
                nc.gpsimd.dma_start(k_all[:, u, :], kT_r[:, u, :])
                nc.gpsimd.dma_start(v_all[:, u, :, :], v_r[:, u, :, :])
            nc.gpsimd.dma_start(q_all[:, :, :QT], qT_r[:, :, :QT])
            qts = [q_all[:, b_, :] for b_ in range(B)]
            kts = [k_all[:, u, :] for u in range(UNITS)]
            vts = [v_all[:, u, :, :] for u in range(UNITS)]
            # W is first needed by the projection one sweep in.
            w_sb = wp.tile([P, D // P, D], bf)
            nc.gpsimd.dma_start(
                w_sb[:], wT_ext.ap().rearrange("(dc p) o -> p dc o", p=P)
            )

            # Exchange bounces: one full chunk per sweep, [8 slices, 130 rows
            # (2 x (64 feat + denom)), 256 tok]. One collective per chunk —
            # each isolated collective pays ~10us of entry/exit overhead, so
            # fewer, bigger ops beat split halves. Distinct tags — a shared
            # tag would alias storage and serialize sweeps.
            a2a_in = [
                dramp.tile(
                    [NCORES, 2 * (DH + 1), CROWS], bf,
                    name=f"a2a_in{i}", tag=f"a2a_in{i}",
                )
                for i in range(NQB)
            ]
            a2a_out = [
                dramp.tile(
                    [NCORES, 2 * (DH + 1), CROWS], bf,
                    name=f"a2a_out{i}", tag=f"a2a_out{i}",
                )
                for i in range(NQB)
            ]

            def attention_block(u, qb):
                """Scores+softmax+AV for unit u, q-block qb; stage the
                [feat+denom, tok] slab to this unit's half-exchange buffer.
                Returns the last AV matmul (ordering anchor)."""
                b_, hi = u // 2, u % 2
                qt2, kt, vt = qts[b_], kts[u], vts[u]
                npairs = 2 * qb + 2
                attn_tiles = []
                for g in range(npairs):
                    ps = pscore.tile([P, 2, QBLK], f32, tag="ps")
                    at = attnp.tile([P, 2, QBLK], bf, tag="attn")
                    for r in range(2):
                        kc = 2 * g + r
                        i = kc - 4 * qb
                        off = i * P if i > 0 else 0
                        nc.tensor.matmul(
                            ps[:, r, off:QBLK],
                            lhsT=kt[:, kc * P : (kc + 1) * P],
                            rhs=qt2[:, qb * QBLK + off : (qb + 1) * QBLK],
                            start=True,
                            stop=True,
                        )
                    # The last diagonal pair (kc = 4qb+2, 4qb+3) only has valid
                    # scores in columns 256:512 — exp'ing the full tile wastes
                    # ~40% of the op on ScalarE, the bottleneck engine.
                    if g == 2 * qb + 1:
                        nc.scalar.activation(
                            at[:, :, 2 * P : QBLK], ps[:, :, 2 * P : QBLK], Exp, scale=0.125
                        )
                    else:
                        nc.scalar.activation(at[:], ps[:], Exp, scale=0.125)
                    for r in range(2):
                        kc = 2 * g + r
                        i = kc - 4 * qb
                        if i >= 0:
                            sl = at[:, r, i * P : (i + 1) * P]
                            nc.vector.tensor_mul(sl, sl, m01[:])
                    attn_tiles.append(at)

                # AV, v stationary: one PSUM accumulation group [dh+1, 512]
                # per q-block. Ascending kc: the first matmul covers the full
                # column range (clears has_written), diagonal chunks then
                # accumulate into their valid suffix only.
                nkc = 4 * qb + 4
                po = pav.tile([DH + 1, QBLK], f32, tag="pav")
                last_av = None
                for kc in range(nkc):
                    g, r = kc // 2, kc % 2
                    i = kc - 4 * qb
                    off = i * P if i > 0 else 0
                    last_av = nc.tensor.matmul(
                        po[:, off:QBLK],
                        lhsT=vt[:, kc, :],
                        rhs=attn_tiles[g][:, r, off:QBLK],
                        start=(kc == 0),
                        stop=(kc == nkc - 1),
                    )
                av_sb = avsp.tile([DH + 1, QBLK], bf, tag="avs")
                nc.vector.tensor_copy(av_sb[:], po[:])
                # Slice halves: tokens (q-tiles 4qb+{0,1}) -> slice 2b, tokens
                # (4qb+{2,3}) -> slice 2b+1; feature rows + denom row together
                # in the unit's 65-row band of the payload.
                dst = a2a_in[qb][b_ * 2 : b_ * 2 + 2, hi * (DH + 1) : (hi + 1) * (DH + 1), :]
                nc.sync.dma_start(
                    dst.rearrange("c f t -> f c t"),
                    av_sb.rearrange("f (c t) -> f c t", c=2),
                )
                return last_av

            def exchange(qb):
                nc.gpsimd.collective_compute(
                    "AllToAll",
                    mybir.AluOpType.bypass,
                    replica_groups=[list(range(NCORES))],
                    ins=[a2a_in[qb].opt()],
                    outs=[a2a_out[qb].opt()],
                )

            proj_at = {}

            def load_chunk(qb):
                """Plain-DMA loads of chunk qb's received payload into the
                feature-major projection tile + denominator rows. Emitted
                mid-next-sweep, before that sweep's (end-of-sweep) exchange,
                so Tile's conservative collective-clock threshold binds it to
                exchange qb only."""
                at_c = atcp.tile([P, NCORES, CROWS], bf, tag="atc")
                den = denp.tile([2 * NCORES, CROWS], bf, tag="den")
                proj_at[qb] = (at_c, den)
                src = a2a_out[qb]
                # On the GPSIMD SWDGE queue: these wait on the collective, and
                # on the SP queue that wait would head-of-line-block the next
                # sweep's stage DMAs and push every later exchange out.
                for h in range(2):
                    nc.gpsimd.dma_start(
                        at_c[h * DH : (h + 1) * DH, :, :],
                        src[:, h * (DH + 1) : h * (DH + 1) + DH, :].rearrange(
                            "s f t -> f s t"
                        ),
                    )
                    nc.gpsimd.dma_start(
                        den[h * NCORES : (h + 1) * NCORES, :],
                        src[:, h * (DH + 1) + DH : (h + 1) * (DH + 1), :].rearrange(
                            "s o t -> (s o) t"
                        ),
                    )

            def normalize_chunk(qb):
                """Reciprocal the 16 denominator rows, replicate them across
                partitions with a 0-stride-AP DMA, normalize in one DVE op."""
                at_c, den = proj_at[qb]
                rec = denp.tile([2 * NCORES, CROWS], bf, tag="rec")
                with nc.allow_low_precision(reason="bf16 softmax denominators"):
                    nc.vector.reciprocal(rec[:], den[:])
                # SBUF APs need a nonzero partition stride, so bounce the 16
                # reciprocal rows through DRAM and replicate on the way back
                # with a 0-stride source dim.
                rec_d = dramp.tile(
                    [2 * NCORES, CROWS], bf, name=f"rec_d{qb}", tag=f"rec_d{qb}"
                )
                nc.gpsimd.dma_start(rec_d[:], rec[:])
                dr = drp.tile([P, NCORES, CROWS], bf, tag="dr")
                for h in range(2):
                    nc.gpsimd.dma_start(
                        dr[h * DH : (h + 1) * DH, :, :],
                        rec_d[h * NCORES : (h + 1) * NCORES, :].partition_broadcast(DH),
                    )
                at_n = atnp.tile([P, NCORES, CROWS], bf, tag="atn")
                nc.vector.tensor_mul(at_n[:], at_c[:], dr[:])
                proj_at[qb] = at_n

            def emit_proj_group(qb, tl, order_after):
                at_n = proj_at[qb]
                ot = osb.tile([P, D], f32, tag="osb")
                for oc in range(2):
                    pp = pproj.tile([P, 512], f32, tag="pp")
                    for dc in range(D // P):
                        mm = nc.tensor.matmul(
                            pp[:],
                            lhsT=at_n[:, dc, tl * P : (tl + 1) * P],
                            rhs=w_sb[:, dc, oc * 512 : (oc + 1) * 512],
                            start=(dc == 0),
                            stop=(dc == D // P - 1),
                        )
                        if dc == 0 and order_after is not None:
                            add_dep_helper(mm.ins, order_after.ins, False,
                                           "keep proj matmuls after attention")
                    nc.vector.tensor_copy(ot[:, oc * 512 : (oc + 1) * 512], pp[:])
                row = qb * CROWS + tl * P
                nc.sync.dma_start(out_ext.ap()[row : row + P, :], ot[:])

            # Sweeps. Chunk qb exchanges once at sweep end; its at-load +
            # normalization land mid-next-sweep (once the collective is
            # surely done) and its projection in that sweep's late phase.
            # The LAST sweep's pending projections are deferred to the tail,
            # where they overlap the final exchange's flight time.
            pending = []
            prev = None
            last_si = len(SWEEP_ORDER) - 1
            for si, qb in enumerate(SWEEP_ORDER):
                last = si == last_si
                for pos, u in enumerate(UNIT_ORDER):
                    anchor = attention_block(u, qb)
                    # In the last (shortest) sweep the previous exchange is
                    # still in flight: emitting its at-load mid-sweep would
                    # head-of-line-block this sweep's stage DMAs on the SP
                    # queue and delay the final exchange. Defer to the tail.
                    if pos == 4 and prev is not None and not last:
                        load_chunk(prev)
                    if pos == 5 and prev is not None and not last:
                        normalize_chunk(prev)
                        pending += [(prev, 0), (prev, 1)]
                    # Pops late (pos 7, then next sweep's 1-2): the previous
                    # exchange only completes ~60% into this sweep, and an
                    # early proj matmul waiting on it stalls the in-order PE
                    # queue (and with it the exp pipeline).
                    if pending and (
                        (pos in (1, 2) and pending[0][0] != prev)
                        or (pos == 7 and not last)
                    ):
                        pqb, ptl = pending.pop(0)
                        emit_proj_group(pqb, ptl, order_after=anchor)
                if last:
                    # Penultimate chunk's at-load: after every stage DMA of
                    # this sweep (no SP head-of-line risk for the final
                    # exchange) but BEFORE the final exchange's emission, so
                    # the collective clock binds it to its own exchange.
                    load_chunk(prev)
                exchange(qb)
                prev = qb
            # Tail: the penultimate chunk normalizes + projects during the
            # final exchange's flight; then the last chunk lands and projects.
            normalize_chunk(SWEEP_ORDER[-2])
            pending += [(SWEEP_ORDER[-2], 0), (SWEEP_ORDER[-2], 1)]
            for pqb, ptl in pending:
                emit_proj_group(pqb, ptl, order_after=None)
            load_chunk(prev)
            normalize_chunk(prev)
            for ptl in range(2):
                emit_proj_group(prev, ptl, order_after=None)

    nc.compile()
    return nc


def _shard_inputs(q, k, v):
    """Build the 8 per-core input maps (bf16, attention-friendly layouts)."""
    qh = np.ascontiguousarray(q.reshape(B, S, H, DH))
    kh = np.ascontiguousarray(k.reshape(B, S, H, DH))
    vh = np.ascontiguousarray(v.reshape(B, S, H, DH))
    in_maps = []
    for c in range(NCORES):
        qT = np.zeros((UNITS // 2, P, S), dtype=BF16)
        kTz = np.zeros((UNITS, P, S), dtype=BF16)
        vv = np.empty((UNITS, P, NKC, DH + 1), dtype=BF16)
        vv[:, :, :, DH] = 1.0
        for b_ in range(B):
            for hi in range(2):
                h = 2 * c + hi
                u = b_ * 2 + hi
                qT[b_, hi * DH : (hi + 1) * DH, :] = qh[b_, :, h, :].T.astype(BF16)
                kTz[u, hi * DH : (hi + 1) * DH, :] = kh[b_, :, h, :].T.astype(BF16)
                vv[u, :, :, 0:DH] = (
                    vh[b_, :, h, :].reshape(NKC, P, DH).transpose(1, 0, 2).astype(BF16)
                )
        in_maps.append(
            {"qT": qT, "kTz": kTz, "v": vv.reshape(UNITS, P, NKC * (DH + 1))}
        )
    return in_maps


def _run(q, k, v, W, trace=False):
    if "nc" not in _CACHE:
        _CACHE["nc"] = _build()
    nc = _CACHE["nc"]
    in_maps = _shard_inputs(q, k, v)
    wT = np.ascontiguousarray(W.T).astype(BF16)
    for m in in_maps:
        m["wT"] = wT
    res = run_bass_kernel_spmd(nc, in_maps, core_ids=list(range(NCORES)), trace=trace)
    out = np.empty((B, S, D), dtype=np.float32)
    for c in range(NCORES):
        b_ = c // 2
        oc = res.results[c]["out"]  # [1024, 1024]: rows qb*256 + jj*128 + p
        for qb in range(NQB):
            for jj in range(2):
                qt = 4 * qb + 2 * (c % 2) + jj
                out[b_, qt * P : (qt + 1) * P, :] = oc[
                    qb * CROWS + jj * P : qb * CROWS + (jj + 1) * P
                ]
    return out, res


def kernel(q, k, v, W, b, mask):
    q = np.asarray(q, dtype=np.float32)
    k = np.asarray(k, dtype=np.float32)
    v = np.asarray(v, dtype=np.float32)
    W = np.asarray(W, dtype=np.float32)
    # b is spec'd all-zero and mask all-zero (no padded keys); the causal mask
    # is applied on-device.
    out, _ = _run(q, k, v, W, trace=False)
    return out


def kernel_profiled(q, k, v, W, b, mask):
    out, res = _run(
        np.asarray(q, np.float32),
        np.asarray(k, np.float32),
        np.asarray(v, np.float32),
        np.asarray(W, np.float32),
        trace=True,
    )
    return out, res


# revision 38
# speedup vs baseline: 1.0805x; 1.0805x over previous
"""Distributed causal multi-head attention + output projection for TRN2 (8 NeuronCores).

Problem: q,k,v [4, 2048, 1024] f32, W [1024, 1024], b zeros, mask zeros (no padding).
  out = proj(softmax(causal(q@k.T/8)) @ v) @ W.T + b

Sharding: head-parallel attention + token-parallel projection, glued by 8-way
AllToAll exchanges of the attention outputs (bf16).
  - Core c computes attention for heads {2c, 2c+1} over all 4 batches
    (8 (batch, head) units/core, identical causal structure on every core -> SPMD-uniform).
  - Core j projects the 1024 tokens {batch j//2, q-tiles 4qb+2*(j%2)+{0,1} for qb 0..3}.
  - Sweeps DESCEND qb (3..0): the collective subsystem's ~65us entry barrier
    (NEFF launch skew + ncfw init — no collective completes before ~70us)
    hides under the two big sweeps, and all bulk input DMA lands during sweep
    3's compute. One full exchange per sweep (each isolated collective pays
    ~8-10us of entry/exit overhead, so fewer/bigger ops beat split halves); a
    tiny warm-up collective absorbs the cold-firmware first-op penalty (~5x).
    Chunk qb's at-load+normalize run mid-sweep qb-1, its projection late in
    that sweep; only the last chunk's ~10us exchange + projection is
    tail-exposed, with the penultimate chunk's projection filling the flight.

Queue discipline (the decisive factor — each hardware DMA queue serializes):
  SP carries only the latency-critical stage DMAs + output stores; every bulk
  or collective-gated transfer (input round-2, W, chunk at-loads, denominator
  broadcasts) issues on the GPSIMD SWDGE queue, so nothing head-of-line-blocks
  the stage DMAs that gate exchange triggers.

Dataflow per unit/q-block:
  QK on PE (k-chunk stationary, q moving 512-wide) -> exp on ScalarE (PSUM
  source, causal tiles trimmed; ScalarE is the bottleneck engine at ~165us) ->
  diagonal-tile multiplicative mask on DVE -> AV on PE with V STATIONARY
  (output [dh+1, 512] in PSUM, ONE accumulation group per q-block — vs
  attention-stationary AV this halves matmul count and cuts 1088 LDWEIGHTS;
  ones-column in v gives the softmax denominator as row 64) -> DVE copy to
  SBUF bf16 -> single stage DMA into the exchange buffer in [feat, tok]
  layout (denominator row included).
  The receiver loads [feat, (src, tok)] tiles with PLAIN DMAs (no transposes
  anywhere — payload is already feature-major), reciprocals the 16 denominator
  rows in one DVE op, replicates them across partitions via a 0-stride-AP DMA
  through a DRAM bounce, normalizes with one tensor_tensor per chunk, and
  runs the projection (at stationary, W moving 512-wide).
"""

import sys

sys.path.insert(0, "/opt/trn_rl_repo")

import numpy as np
import ml_dtypes

import concourse.bass as bass  # noqa: F401
import concourse.mybir as mybir
from concourse import bacc
from concourse.bass_utils import run_bass_kernel_spmd
from concourse.tile import TileContext
from concourse.masks import make_upper_triangular
from bass_rust import add_dep_helper

B, S, D, H, DH = 4, 2048, 1024, 16, 64
P = 128
NCORES = 8
UNITS = 8          # (batch, local head) pairs per core
QBLK = 512         # q columns per score block
NQB = S // QBLK    # 4
NKC = S // P       # 16 key chunks
TOK = (B * S) // NCORES  # 1024 tokens projected per core
CROWS = 256        # token rows per core per exchange chunk

# Descending: the ~65us collective-subsystem entry barrier (NEFF launch
# barrier + ncfw init — no collective completes before ~70us) hides under the
# two big sweeps, and all input DMA traffic lands during sweep 3's 60us+ of
# compute, leaving the second half's exchanges an uncontended fabric.
SWEEP_ORDER = [3, 2, 1, 0]
UNIT_ORDER = [0, 2, 4, 6, 1, 3, 5, 7]  # evens feed half-exchange A, odds B

BF16 = ml_dtypes.bfloat16

_CACHE = {}


def _build():
    bf = mybir.dt.bfloat16
    f32 = mybir.dt.float32
    Exp = mybir.ActivationFunctionType.Exp

    nc = bacc.Bacc("TRN2", target_bir_lowering=False, debug=False, num_devices=NCORES)

    kT_ext = nc.declare_dram_parameter("kTz", [UNITS, P, S], bf, isOutput=False)
    qT_ext = nc.declare_dram_parameter("qT", [UNITS // 2, P, S], bf, isOutput=False)
    v_ext = nc.declare_dram_parameter("v", [UNITS, P, NKC * (DH + 1)], bf, isOutput=False)
    wT_ext = nc.declare_dram_parameter("wT", [D, D], bf, isOutput=False)
    out_ext = nc.declare_dram_parameter("out", [TOK, D], f32, isOutput=True)

    with TileContext(nc) as tc:
        with (
            tc.tile_pool(name="const", bufs=1) as constp,
            tc.tile_pool(name="q", bufs=1) as qp,
            tc.tile_pool(name="k", bufs=1) as kp,
            tc.tile_pool(name="v", bufs=1) as vp,
            tc.tile_pool(name="attn", bufs=22) as attnp,
            tc.tile_pool(name="avs", bufs=10) as avsp,
            tc.tile_pool(name="atc", bufs=2) as atcp,
            tc.tile_pool(name="atn", bufs=2) as atnp,
            tc.tile_pool(name="dr", bufs=2) as drp,
            tc.tile_pool(name="den", bufs=2) as denp,
            tc.tile_pool(name="w", bufs=1) as wp,
            tc.tile_pool(name="osb", bufs=2) as osb,
            tc.tile_pool(name="dram", bufs=1, space="DRAM") as dramp,
            tc.tile_pool(name="pscore", bufs=2, space="PSUM") as pscore,
            tc.tile_pool(name="pav", bufs=2, space="PSUM") as pav,
            tc.tile_pool(name="pproj", bufs=2, space="PSUM") as pproj,
        ):
            # Multiplicative causal mask for diagonal tiles, [k, q] layout:
            # m01[kk, qq] = 1.0 iff qq >= kk.
            m01 = constp.tile([P, P], bf)
            make_upper_triangular(nc, m01[:], val=1.0, diag=True)

            # Resident q/k/v in fused tiles (unit on a free dim). Sweep qb=3
            # needs unit u's FULL k/v and q columns 1536:2048, so load
            # per-unit in sweep unit order: the first unit's tensors on the
            # SP queue (compute starts ~3us in), everything else on the
            # GPSIMD SWDGE queue so the ~10MB of bulk transfers never
            # serialize ahead of stage DMAs on the SP hardware queue.
            k_all = kp.tile([P, UNITS, S], bf)
            q_all = qp.tile([P, B, S], bf)
            v_all = vp.tile([P, UNITS, NKC, DH + 1], bf)
            kT_r = kT_ext.ap().rearrange("u p s -> p u s")
            qT_r = qT_ext.ap().rearrange("b p s -> p b s")
            v_r = v_ext.ap().rearrange("u p (c d) -> p u c d", d=DH + 1)
            QT = (NQB - 1) * QBLK
            u0 = UNIT_ORDER[0]
            nc.sync.dma_start(k_all[:, u0, :QBLK], kT_r[:, u0, :QBLK])
            nc.sync.dma_start(q_all[:, 0:1, QT:], qT_r[:, 0:1, QT:])
            nc.sync.dma_start(v_all[:, u0, 0:4, :], v_r[:, u0, 0:4, :])
            nc.sync.dma_start(k_all[:, u0, QBLK:], kT_r[:, u0, QBLK:])
            nc.sync.dma_start(q_all[:, 1:, QT:], qT_r[:, 1:, QT:])
            nc.sync.dma_start(v_all[:, u0, 4:, :], v_r[:, u0, 4:, :])
            # Tiny warm-up collective, triggered before the bulk round-2
            # issues occupy the GPSIMD queue: the first collective after the
            # entry barrier otherwise runs ~5x slower (cold firmware path),
            # which in run timings turned a 10us exchange into 52us.
            a2a_wi = dramp.tile([NCORES, 64], bf, name="a2a_wi", tag="a2a_wi")
            a2a_wo = dramp.tile([NCORES, 64], bf, name="a2a_wo", tag="a2a_wo")
            nc.gpsimd.collective_compute(
                "AllToAll",
                mybir.AluOpType.bypass,
                replica_groups=[list(range(NCORES))],
                ins=[a2a_wi.opt()],
                outs=[a2a_wo.opt()],
            )
            for u in UNIT_ORDER[1:]:
                nc.gpsimd.dma_start(k_all[:, u, :], kT_r[:, u, :])
                nc.gpsimd.dma_start(v_all[:, u, :, :], v_r[:, u, :, :])
            nc.gpsimd.dma_start(q_all[:, :, :QT], qT_r[:, :, :QT])
            qts = [q_all[:, b_, :] for b_ in range(B)]
            kts = [k_all[:, u, :] for u in range(UNITS)]
            vts = [v_all[:, u, :, :] for u in range(UNITS)]
            # W is first needed by the projection one sweep in.
            w_sb = wp.tile([P, D // P, D], bf)
            nc.gpsimd.dma_start(
                w_sb[:], wT_ext.ap().rearrange("(dc p) o -> p dc o", p=P)
            )

            # Exchange bounces: one full chunk per sweep, [8 slices, 130 rows
            # (2 x (64 feat + denom)), 256 tok]. One collective per chunk —
            # each isolated collective pays ~10us of entry/exit overhead, so
            # fewer, bigger ops beat split halves. Distinct tags — a shared
            # tag would alias storage and serialize sweeps.
            a2a_in = [
                dramp.tile(
                    [NCORES, 2 * (DH + 1), CROWS], bf,
                    name=f"a2a_in{i}", tag=f"a2a_in{i}",
                )
                for i in range(NQB)
            ]
            a2a_out = [
                dramp.tile(
                    [NCORES, 2 * (DH + 1), CROWS], bf,
                    name=f"a2a_out{i}", tag=f"a2a_out{i}",
                )
                for i in range(NQB)
            ]

            def attention_block(u, qb):
                """Scores+softmax+AV for unit u, q-block qb; stage the
                [feat+denom, tok] slab to this unit's half-exchange buffer.
                Returns the last AV matmul (ordering anchor)."""
                b_, hi = u // 2, u % 2
                qt2, kt, vt = qts[b_], kts[u], vts[u]
                npairs = 2 * qb + 2
                attn_tiles = []
                for g in range(npairs):
                    ps = pscore.tile([P, 2, QBLK], f32, tag="ps")
                    at = attnp.tile([P, 2, QBLK], bf, tag="attn")
                    for r in range(2):
                        kc = 2 * g + r
                        i = kc - 4 * qb
                        off = i * P if i > 0 else 0
                        nc.tensor.matmul(
                            ps[:, r, off:QBLK],
                            lhsT=kt[:, kc * P : (kc + 1) * P],
                            rhs=qt2[:, qb * QBLK + off : (qb + 1) * QBLK],
                            start=True,
                            stop=True,
                        )
                    # The last diagonal pair (kc = 4qb+2, 4qb+3) only has valid
                    # scores in columns 256:512 — exp'ing the full tile wastes
                    # ~40% of the op on ScalarE, the bottleneck engine.
                    if g == 2 * qb + 1:
                        nc.scalar.activation(
                            at[:, :, 2 * P : QBLK], ps[:, :, 2 * P : QBLK], Exp, scale=0.125
                        )
                    else:
                        nc.scalar.activation(at[:], ps[:], Exp, scale=0.125)
                    for r in range(2):
                        kc = 2 * g + r
                        i = kc - 4 * qb
                        if i >= 0:
                            sl = at[:, r, i * P : (i + 1) * P]
                            nc.vector.tensor_mul(sl, sl, m01[:])
                    attn_tiles.append(at)

                # AV, v stationary: one PSUM accumulation group [dh+1, 512]
                # per q-block. Ascending kc: the first matmul covers the full
                # column range (clears has_written), diagonal chunks then
                # accumulate into their valid suffix only.
                nkc = 4 * qb + 4
                po = pav.tile([DH + 1, QBLK], f32, tag="pav")
                last_av = None
                for kc in range(nkc):
                    g, r = kc // 2, kc % 2
                    i = kc - 4 * qb
                    off = i * P if i > 0 else 0
                    last_av = nc.tensor.matmul(
                        po[:, off:QBLK],
                        lhsT=vt[:, kc, :],
                        rhs=attn_tiles[g][:, r, off:QBLK],
                        start=(kc == 0),
                        stop=(kc == nkc - 1),
                    )
                av_sb = avsp.tile([DH + 1, QBLK], bf, tag="avs")
                nc.vector.tensor_copy(av_sb[:], po[:])
                # Slice halves: tokens (q-tiles 4qb+{0,1}) -> slice 2b, tokens
                # (4qb+{2,3}) -> slice 2b+1; feature rows + denom row together
                # in the unit's 65-row band of the payload.
                dst = a2a_in[qb][b_ * 2 : b_ * 2 + 2, hi * (DH + 1) : (hi + 1) * (DH + 1), :]
                nc.sync.dma_start(
                    dst.rearrange("c f t -> f c t"),
                    av_sb.rearrange("f (c t) -> f c t", c=2),
                )
                return last_av

            def exchange(qb):
                nc.gpsimd.collective_compute(
                    "AllToAll",
                    mybir.AluOpType.bypass,
                    replica_groups=[list(range(NCORES))],
                    ins=[a2a_in[qb].opt()],
                    outs=[a2a_out[qb].opt()],
                )

            proj_at = {}

            def load_chunk(qb):
                """Plain-DMA loads of chunk qb's received payload into the
                feature-major projection tile + denominator rows. Emitted
                mid-next-sweep, before that sweep's (end-of-sweep) exchange,
                so Tile's conservative collective-clock threshold binds it to
                exchange qb only."""
                at_c = atcp.tile([P, NCORES, CROWS], bf, tag="atc")
                den = denp.tile([2 * NCORES, CROWS], bf, tag="den")
                proj_at[qb] = (at_c, den)
                src = a2a_out[qb]
                # On the GPSIMD SWDGE queue: these wait on the collective, and
                # on the SP queue that wait would head-of-line-block the next
                # sweep's stage DMAs and push every later exchange out.
                for h in range(2):
                    nc.gpsimd.dma_start(
                        at_c[h * DH : (h + 1) * DH, :, :],
                        src[:, h * (DH + 1) : h * (DH + 1) + DH, :].rearrange(
                            "s f t -> f s t"
                        ),
                    )
                    nc.gpsimd.dma_start(
                        den[h * NCORES : (h + 1) * NCORES, :],
                        src[:, h * (DH + 1) + DH : (h + 1) * (DH + 1), :].rearrange(
                            "s o t -> (s o) t"
                        ),
                    )

            def normalize_chunk(qb, dma_eng=None):
                """Reciprocal the 16 denominator rows, replicate them across
                partitions with a 0-stride-AP DMA, normalize in one DVE op.
                dma_eng picks the queue for the bounce/broadcast DMAs:
                gpsimd mid-sweep (SP carries stage DMAs there), SP in the
                tail (the gpsimd queue is blocked behind the final exchange
                trigger's stage-DMA wait, SP is already drained)."""
                if dma_eng is None:
                    dma_eng = nc.gpsimd
                at_c, den = proj_at[qb]
                rec = denp.tile([2 * NCORES, CROWS], bf, tag="rec")
                with nc.allow_low_precision(reason="bf16 softmax denominators"):
                    nc.vector.reciprocal(rec[:], den[:])
                # SBUF APs need a nonzero partition stride, so bounce the 16
                # reciprocal rows through DRAM and replicate on the way back
                # with a 0-stride source dim.
                rec_d = dramp.tile(
                    [2 * NCORES, CROWS], bf, name=f"rec_d{qb}", tag=f"rec_d{qb}"
                )
                dma_eng.dma_start(rec_d[:], rec[:])
                dr = drp.tile([P, NCORES, CROWS], bf, tag="dr")
                for h in range(2):
                    dma_eng.dma_start(
                        dr[h * DH : (h + 1) * DH, :, :],
                        rec_d[h * NCORES : (h + 1) * NCORES, :].partition_broadcast(DH),
                    )
                at_n = atnp.tile([P, NCORES, CROWS], bf, tag="atn")
                nc.vector.tensor_mul(at_n[:], at_c[:], dr[:])
                proj_at[qb] = at_n

            def emit_proj_group(qb, tl, order_after):
                at_n = proj_at[qb]
                ot = osb.tile([P, D], f32, tag="osb")
                for oc in range(2):
                    pp = pproj.tile([P, 512], f32, tag="pp")
                    for dc in range(D // P):
                        mm = nc.tensor.matmul(
                            pp[:],
                            lhsT=at_n[:, dc, tl * P : (tl + 1) * P],
                            rhs=w_sb[:, dc, oc * 512 : (oc + 1) * 512],
                            start=(dc == 0),
                            stop=(dc == D // P - 1),
                        )
                        if dc == 0 and order_after is not None:
                            add_dep_helper(mm.ins, order_after.ins, False,
                                           "keep proj matmuls after attention")
                    nc.vector.tensor_copy(ot[:, oc * 512 : (oc + 1) * 512], pp[:])
                row = qb * CROWS + tl * P
                nc.sync.dma_start(out_ext.ap()[row : row + P, :], ot[:])

            # Sweeps. Chunk qb exchanges once at sweep end; its at-load +
            # normalization land mid-next-sweep (once the collective is
            # surely done) and its projection in that sweep's late phase.
            # The LAST sweep's pending projections are deferred to the tail,
            # where they overlap the final exchange's flight time.
            pending = []
            prev = None
            last_si = len(SWEEP_ORDER) - 1
            for si, qb in enumerate(SWEEP_ORDER):
                last = si == last_si
                for pos, u in enumerate(UNIT_ORDER):
                    anchor = attention_block(u, qb)
                    # In the last (shortest) sweep the previous exchange is
                    # still in flight: emitting its at-load mid-sweep would
                    # head-of-line-block this sweep's stage DMAs on the SP
                    # queue and delay the final exchange. Defer to the tail.
                    if pos == 4 and prev is not None and not last:
                        load_chunk(prev)
                    if pos == 5 and prev is not None and not last:
                        normalize_chunk(prev)
                        pending += [(prev, 0), (prev, 1)]
                    # Pops late (pos 7, then next sweep's 1-2): the previous
                    # exchange only completes ~60% into this sweep, and an
                    # early proj matmul waiting on it stalls the in-order PE
                    # queue (and with it the exp pipeline).
                    if pending and (
                        (pos in (1, 2) and pending[0][0] != prev)
                        or (pos == 7 and not last)
                    ):
                        pqb, ptl = pending.pop(0)
                        emit_proj_group(pqb, ptl, order_after=anchor)
                if last:
                    # Penultimate chunk's at-load: after every stage DMA of
                    # this sweep (no SP head-of-line risk for the final
                    # exchange) but BEFORE the final exchange's emission, so
                    # the collective clock binds it to its own exchange.
                    load_chunk(prev)
                exchange(qb)
                prev = qb
            # Tail: the penultimate chunk normalizes + projects during the
            # final exchange's flight; then the last chunk lands and projects.
            normalize_chunk(SWEEP_ORDER[-2], dma_eng=nc.sync)
            pending += [(SWEEP_ORDER[-2], 0), (SWEEP_ORDER[-2], 1)]
            for pqb, ptl in pending:
                emit_proj_group(pqb, ptl, order_after=None)
            load_chunk(prev)
            normalize_chunk(prev)
            for ptl in range(2):
                emit_proj_group(prev, ptl, order_after=None)

    nc.compile()
    return nc


def _shard_inputs(q, k, v):
    """Build the 8 per-core input maps (bf16, attention-friendly layouts)."""
    qh = np.ascontiguousarray(q.reshape(B, S, H, DH))
    kh = np.ascontiguousarray(k.reshape(B, S, H, DH))
    vh = np.ascontiguousarray(v.reshape(B, S, H, DH))
    in_maps = []
    for c in range(NCORES):
        qT = np.zeros((UNITS // 2, P, S), dtype=BF16)
        kTz = np.zeros((UNITS, P, S), dtype=BF16)
        vv = np.empty((UNITS, P, NKC, DH + 1), dtype=BF16)
        vv[:, :, :, DH] = 1.0
        for b_ in range(B):
            for hi in range(2):
                h = 2 * c + hi
                u = b_ * 2 + hi
                qT[b_, hi * DH : (hi + 1) * DH, :] = qh[b_, :, h, :].T.astype(BF16)
                kTz[u, hi * DH : (hi + 1) * DH, :] = kh[b_, :, h, :].T.astype(BF16)
                vv[u, :, :, 0:DH] = (
                    vh[b_, :, h, :].reshape(NKC, P, DH).transpose(1, 0, 2).astype(BF16)
                )
        in_maps.append(
            {"qT": qT, "kTz": kTz, "v": vv.reshape(UNITS, P, NKC * (DH + 1))}
        )
    return in_maps


def _run(q, k, v, W, trace=False):
    if "nc" not in _CACHE:
        _CACHE["nc"] = _build()
    nc = _CACHE["nc"]
    in_maps = _shard_inputs(q, k, v)
    wT = np.ascontiguousarray(W.T).astype(BF16)
    for m in in_maps:
        m["wT"] = wT
    res = run_bass_kernel_spmd(nc, in_maps, core_ids=list(range(NCORES)), trace=trace)
    out = np.empty((B, S, D), dtype=np.float32)
    for c in range(NCORES):
        b_ = c // 2
        oc = res.results[c]["out"]  # [1024, 1024]: rows qb*256 + jj*128 + p
        for qb in range(NQB):
            for jj in range(2):
                qt = 4 * qb + 2 * (c % 2) + jj
                out[b_, qt * P : (qt + 1) * P, :] = oc[
                    qb * CROWS + jj * P : qb * CROWS + (jj + 1) * P
                ]
    return out, res


def kernel(q, k, v, W, b, mask):
    q = np.asarray(q, dtype=np.float32)
    k = np.asarray(k, dtype=np.float32)
    v = np.asarray(v, dtype=np.float32)
    W = np.asarray(W, dtype=np.float32)
    # b is spec'd all-zero and mask all-zero (no padded keys); the causal mask
    # is applied on-device.
    out, _ = _run(q, k, v, W, trace=False)
    return out


def kernel_profiled(q, k, v, W, b, mask):
    out, res = _run(
        np.asarray(q, np.float32),
        np.asarray(k, np.float32),
        np.asarray(v, np.float32),
        np.asarray(W, np.float32),
        trace=True,
    )
    return out, res
